# revision 2
# baseline (speedup 1.0000x reference)
"""Causal self-attention (B=4, T=2048, C=1024, NH=16) on 8 trn2 NeuronCores.

Sharding: core = (head_group hg in {0,1}) x (batch b in {0..3}).
Each core computes qkv projection + attention + partial output projection for
its 8 heads of its batch; host sums the two head-group partials per batch and
adds the output bias.

v3 = v2 + fp8 DoubleRow "3-slot split" for the qkv and output projections.
  - A matmul instruction costs out_free_size x cycles_per_row; fp8e4/e5 with
    perf_mode=DoubleRow runs at 0.5 cycles/row and contracts TWO 128-row
    k-tiles per instruction (lhsT [128,2,M], rhs [128,2,N]).  Writing
    X = Xh + Xl and W = Wh + Wl (each an e4m3 pair: hi = e4(x), lo =
    e4(x - hi), together ~9 mantissa bits > bf16's 8), the product
    X@W ~= Xh@Wh + Xh@Wl + Xl@Wh needs 3 slot-products per k-tile = 1.5
    DoubleRow instructions per k-tile pair = 0.75x the bf16 PE cost.
    Measured end-to-end rel-err 4.3e-3 vs bf16's 5.6e-3 (the e4m3 pair is
    slightly MORE precise than bf16).
  - Scale scheme keeps every fp8 operand in e4m3 normal range with all
    compensation factors exact powers of two: x' = 16x, W' = 64Wqkv =>
    PSUM q/k/v are 1024x; bias ships 1024x; scores are 2^20 x so the exp
    scale is 0.125*2^-20; the AV ones-column is 64.0 so o = acc*dinv comes
    out 16x; Wproj' = 64Wproj => y partials are 1024x, divided on the host.
  - qkv/V weights and x ship as host-packed hi/lo e4m3 pairs in DR-friendly
    row order (slot pairs contiguous), so SBUF tiles load with plain DMAs:
    same total bytes as the bf16 v2 (hi+lo = 2 bytes/elem).
  - o (= 16x true o, bf16) transposes to oT exactly as v2 (one
    dma_start_transpose per 128-query chunk), then splits on DVE into
    oT_hi = e4(oT), oT_lo = e4(oT - oT_hi) for the deferred DR projection.
  - scores and AV stay bf16: the score contraction is only 64 (no k-tile
    pair to fuse) and AV's pt residual would need a second exp pass.

Attention core (unchanged from v2):
  - q/k computed transposed (head_size on partitions); V in natural
    [token, feat] layout; V bias folded into the host-side output bias.
  - S^T = K @ Q^T per (head, 128-key block, 128-query chunk); 8 causal
    blocks packed in one [128, 1024] PSUM tile so one ScalarE Exp covers
    them.  Causal mask = one 0/1 multiply per diagonal block on GpSimd.
  - AV runs query-on-partitions: acc[q, 0:65] += pt_slice^T @ [V | 64];
    the softmax denominator arrives as a per-partition scalar -> DVE
    reciprocal + tensor_scalar_mul.
  - output projections deferred to the end of the program as PE fill for
    the ACT-bound late tiles.
  - a few throwaway warmup matmuls burn the PE p-state ramp.
Cost-model span: see test.py (v2 was 203588 ns/core; PE busy was 97.5%).
"""

import sys

sys.path.insert(0, "/opt/trn_rl_repo")

import numpy as np

import concourse.bacc as bacc
import concourse.bass as bass
import concourse.mybir as mybir
from concourse.bass_utils import run_bass_kernel_spmd
from concourse.tile import TileContext

B, T, C, NH = 4, 2048, 1024, 16
HS = C // NH          # 64
HGF = 512             # features per head group (8 heads x 64)
QT = 256              # query tile (S stage)
NKT = T // 128        # 16 key tiles
F32 = mybir.dt.float32
BF16 = mybir.dt.bfloat16
E4 = mybir.dt.float8e4
DR = mybir.MatmulPerfMode.DoubleRow
Exp = mybir.ActivationFunctionType.Exp

SA = 16.0             # x scale
SB = 64.0             # weight scale
EXP_SCALE = 0.125 / float(SA * SA * SB * SB)   # 0.125 * 2^-20
ONES_VAL = 64.0       # AV denominator column value => o = 16x true o
OUT_DIV = SA * SB     # host divides y partials by 1024


def build_kernel():
    nc = bacc.Bacc(None, target_bir_lowering=False)
    # packed x: row = 128*pr + p, col = 1024*n + 512*two + t
    xh = nc.dram_tensor("xh", (512, 4096), E4, kind="ExternalInput")
    xl = nc.dram_tensor("xl", (512, 4096), E4, kind="ExternalInput")
    # packed q/k weights: row = 128*m + p (m 0..3 q, 4..7 k),
    # col = 256*pr + 128*two + c
    wqkh = nc.dram_tensor("wqkh", (1024, 1024), E4, kind="ExternalInput")
    wqkl = nc.dram_tensor("wqkl", (1024, 1024), E4, kind="ExternalInput")
    # packed v weights (moving layout): row = 128*pr + p, col = 512*two + c
    wvh = nc.dram_tensor("wvh", (512, 1024), E4, kind="ExternalInput")
    wvl = nc.dram_tensor("wvl", (512, 1024), E4, kind="ExternalInput")
    bqk = nc.dram_tensor("bqk", (128, 8), F32, kind="ExternalInput")
    # packed proj weights: row = 128*g + p, col = 1024*two + c
    wph = nc.dram_tensor("wph", (256, 2048), E4, kind="ExternalInput")
    wpl = nc.dram_tensor("wpl", (256, 2048), E4, kind="ExternalInput")
    mask01 = nc.dram_tensor("mask01", (128, 128), BF16, kind="ExternalInput")
    y = nc.dram_tensor("y", (T, C), F32, kind="ExternalOutput")

    with TileContext(nc) as tc:
        with (
            tc.tile_pool(name="outer", bufs=1) as outer,
            tc.tile_pool(name="work", bufs=1) as work,
            tc.tile_pool(name="psum", bufs=1, space="PSUM") as psum,
        ):
            # ---- PE p-state warmup ----
            warm = outer.tile([128, 512], BF16, name="warm")
            nc.vector.memset(warm, 0.0)
            for wi in range(5):
                wtag, wbufs = ("py", 1) if wi % 2 == 0 else ("mm", 2)
                wps = psum.tile([128, 512], F32, tag=wtag, bufs=wbufs,
                                name=f"warm{wi}")
                nc.tensor.matmul(wps, warm[:, 0:128], warm,
                                 start=True, stop=True)

            bias_all = outer.tile([128, 8], F32, name="bias_all")
            mask_b = outer.tile([128, 128], BF16, name="mask_b")
            # q/k weight m-tiles [p, pr, two, c]; q part (m<4) loads first
            # on the idle scalar queue so the first qkv chain isn't blocked
            wqk_h = [outer.tile([128, 4, 2, 128], E4, name=f"wqh{m}")
                     for m in range(8)]
            wqk_l = [outer.tile([128, 4, 2, 128], E4, name=f"wql{m}")
                     for m in range(8)]
            for m in range(4):
                nc.scalar.dma_start(
                    wqk_h[m], wqkh[128 * m:128 * m + 128, :].rearrange(
                        "p (pr two c) -> p pr two c", two=2, c=128))
                nc.scalar.dma_start(
                    wqk_l[m], wqkl[128 * m:128 * m + 128, :].rearrange(
                        "p (pr two c) -> p pr two c", two=2, c=128))
            wv_h = [outer.tile([128, 2, 512], E4, name=f"wvh{pr}")
                    for pr in range(4)]
            wv_l = [outer.tile([128, 2, 512], E4, name=f"wvl{pr}")
                    for pr in range(4)]
            wp_h = [outer.tile([128, 2, 1024], E4, name=f"wph{g}")
                    for g in range(2)]
            wp_l = [outer.tile([128, 2, 1024], E4, name=f"wpl{g}")
                    for g in range(2)]

            k_t = [outer.tile([128, T], BF16, name=f"k{i}") for i in range(4)]
            q_sb = [outer.tile([128, T], BF16, name=f"q{i}") for i in range(4)]
            # v_store[i]: [key-tile 128, 8*65]; per head h cols 65h:65h+64 are
            # V features (1024x), col 65h+64 is ONES_VAL (softmax denominator)
            v_store = [outer.tile([128, 8 * 65], BF16, name=f"v{i}")
                       for i in range(NKT)]
            for i in range(NKT):
                nc.vector.memset(
                    v_store[i].rearrange("p (g c) -> p g c", c=65)[:, :, 64:65],
                    ONES_VAL,
                )

            oT_all = []
            for n in range(4):  # 512-token chunks
                # ---- x DMAs for chunk n: hi/lo per k-tile pair ----
                x_h, x_l = [], []
                for pr in range(4):
                    xt_h = work.tile([128, 2, 512], E4, tag=f"xh{pr}", bufs=2,
                                     name=f"xh{n}_{pr}")
                    xt_l = work.tile([128, 2, 512], E4, tag=f"xl{pr}", bufs=2,
                                     name=f"xl{n}_{pr}")
                    # chunk 0 split across two queues so all 8 tiles beat the
                    # first qkv accumulation chain
                    heng = nc.sync if (n == 0 and pr >= 2) else nc.gpsimd
                    leng = nc.sync if (n == 0) else nc.gpsimd
                    heng.dma_start(
                        xt_h, xh[128 * pr:128 * pr + 128,
                                 1024 * n:1024 * n + 1024].rearrange(
                            "p (two c) -> p two c", two=2))
                    leng.dma_start(
                        xt_l, xl[128 * pr:128 * pr + 128,
                                 1024 * n:1024 * n + 1024].rearrange(
                            "p (two c) -> p two c", two=2))
                    x_h.append(xt_h)
                    x_l.append(xt_l)
                if n == 0:
                    # late-needed loads, queued behind the first x chunk
                    for pr in range(4):
                        veng = nc.gpsimd if pr < 2 else nc.sync
                        veng.dma_start(
                            wv_h[pr], wvh[128 * pr:128 * pr + 128, :].rearrange(
                                "p (two c) -> p two c", two=2))
                        veng.dma_start(
                            wv_l[pr], wvl[128 * pr:128 * pr + 128, :].rearrange(
                                "p (two c) -> p two c", two=2))
                    nc.gpsimd.dma_start(bias_all, bqk[:, :])
                    nc.gpsimd.dma_start(mask_b, mask01[:, :])
                    for m in range(4, 8):
                        nc.sync.dma_start(
                            wqk_h[m], wqkh[128 * m:128 * m + 128, :].rearrange(
                                "p (pr two c) -> p pr two c", two=2, c=128))
                        nc.sync.dma_start(
                            wqk_l[m], wqkl[128 * m:128 * m + 128, :].rearrange(
                                "p (pr two c) -> p pr two c", two=2, c=128))
                    for g in range(2):
                        nc.sync.dma_start(
                            wp_h[g], wph[128 * g:128 * g + 128, :].rearrange(
                                "p (two c) -> p two c", two=2))
                        nc.sync.dma_start(
                            wp_l[g], wpl[128 * g:128 * g + 128, :].rearrange(
                                "p (two c) -> p two c", two=2))

                # per head pair p: q then k m-tiles, then the pair's
                # attention for BOTH query tiles; V rides inside pair 0.
                o_sb = {
                    j: [
                        work.tile([128, HGF], BF16, tag=f"os{s}", bufs=4,
                                  name=f"o{j}_{s}")
                        for s in range(2)
                    ]
                    for j in (2 * n, 2 * n + 1)
                }
                for p in range(4):
                    for m in (p, 4 + p):  # q then k, transposed layout
                        ps = psum.tile([128, 512], F32, tag="mm", bufs=2,
                                       name=f"ps{n}_{m}")
                        kk = 0
                        for pr in range(4):
                            for (wt, xt) in ((wqk_h[m], x_h[pr]),
                                             (wqk_l[m], x_h[pr]),
                                             (wqk_h[m], x_l[pr])):
                                nc.tensor.matmul(
                                    ps, wt[:, pr], xt,
                                    start=(kk == 0), stop=(kk == 11),
                                    perf_mode=DR,
                                )
                                kk += 1
                        dst = q_sb[m] if m < 4 else k_t[m - 4]
                        nc.vector.tensor_scalar_add(
                            dst[:, n * 512:(n + 1) * 512], ps,
                            bias_all[:, m:m + 1]
                        )
                    if p == 0:
                        for t4 in range(4):
                            vtag, vbufs = (("acc", 1) if n == 0 and t4 == 2
                                           else ("mm", 2))
                            ps = psum.tile([128, 512], F32, tag=vtag,
                                           bufs=vbufs, name=f"psv{n}_{t4}")
                            kk = 0
                            for pr in range(4):
                                xs_h = x_h[pr][:, :, t4 * 128:(t4 + 1) * 128]
                                xs_l = x_l[pr][:, :, t4 * 128:(t4 + 1) * 128]
                                for (lt, rt) in ((xs_h, wv_h[pr]),
                                                 (xs_h, wv_l[pr]),
                                                 (xs_l, wv_h[pr])):
                                    nc.tensor.matmul(
                                        ps, lt, rt,
                                        start=(kk == 0), stop=(kk == 11),
                                        perf_mode=DR,
                                    )
                                    kk += 1
                            vt = v_store[4 * n + t4]
                            nc.vector.tensor_copy(
                                vt.rearrange("p (g c) -> p g c", c=65)[:, :, 0:64],
                                ps.rearrange("p (g c) -> p g c", c=64),
                            )
                    for j, h in [(2 * n, 2 * p), (2 * n, 2 * p + 1),
                                 (2 * n + 1, 2 * p), (2 * n + 1, 2 * p + 1)]:
                        pair, off = h // 2, 64 * (h % 2)
                        acc2 = psum.tile([128, 512], F32, tag="acc", bufs=1,
                                         name=f"acc{j}_{h}")
                        acc = [acc2[:, 256 * s:256 * s + 65] for s in range(2)]
                        blocks = [(s, i) for s in range(2)
                                  for i in range(2 * j + s + 1)]
                        for g in range((len(blocks) + 7) // 8):
                            grp = blocks[8 * g:8 * g + 8]
                            sg = psum.tile([128, 1024], F32, tag="big",
                                           bufs=2, name=f"sg{j}_{h}_{g}")
                            for bi, (s, i) in enumerate(grp):
                                nc.tensor.matmul(
                                    sg[:, bi * 128:(bi + 1) * 128],
                                    k_t[pair][off:off + 64, i * 128:(i + 1) * 128],
                                    q_sb[pair][off:off + 64,
                                               j * QT + s * 128:
                                               j * QT + s * 128 + 128],
                                    start=True,
                                    stop=True,
                                )
                            pt = work.tile([128, 1024], BF16, tag="pt",
                                           bufs=6, name=f"pt{j}_{h}_{g}")
                            nc.scalar.activation(
                                pt[:, :len(grp) * 128], sg[:, :len(grp) * 128],
                                Exp, scale=EXP_SCALE
                            )
                            for bi, (s, i) in enumerate(grp):
                                if i == 2 * j + s:  # diagonal triangle
                                    nc.gpsimd.tensor_mul(
                                        pt[:, bi * 128:(bi + 1) * 128],
                                        pt[:, bi * 128:(bi + 1) * 128],
                                        mask_b,
                                    )
                            for bi, (s, i) in enumerate(grp):
                                nc.tensor.matmul(
                                    acc[s],
                                    pt[:, bi * 128:(bi + 1) * 128],
                                    v_store[i][:, 65 * h:65 * h + 65],
                                    start=(i == 0),
                                    stop=(i == 2 * j + s),
                                )
                        for s in range(2):
                            dinv = work.tile([128, 1], F32, tag="dinv", bufs=4,
                                             name=f"di{j}_{h}_{s}")
                            nc.vector.reciprocal(dinv, acc[s][:, 64:65])
                            nc.vector.tensor_scalar_mul(
                                o_sb[j][s][:, 64 * h:64 * h + 64],
                                acc[s][:, 0:64],
                                dinv,
                            )
                        del acc2
                # o -> proj-ready layout now (frees o_sb): transpose in bf16,
                # then split to e4m3 hi/lo on DVE for the deferred projection
                for j in (2 * n, 2 * n + 1):
                    for s in range(2):
                        oT = work.tile([128, 4, 128], BF16, tag="ot", bufs=4,
                                       name=f"ot{j}_{s}")
                        nc.sync.dma_start_transpose(oT, o_sb[j][s])
                        oT_hi = work.tile([128, 4, 128], E4, tag="oth",
                                          bufs=16, name=f"oth{j}_{s}")
                        oT_lo = work.tile([128, 4, 128], E4, tag="otl",
                                          bufs=16, name=f"otl{j}_{s}")
                        nc.vector.tensor_copy(oT_hi, oT)
                        nc.vector.tensor_tensor(
                            oT_lo, oT, oT_hi, mybir.AluOpType.subtract)
                        oT_all.append((j, s, oT_hi, oT_lo))

            # ---- deferred output projections (fp8 DR 3-slot) ----
            for idx, (j, s, oT_hi, oT_lo) in enumerate(oT_all):
                for nn in range(2):
                    tag = "py" if (2 * idx + nn) % 3 == 0 else "mm"
                    psy = psum.tile([128, 512], F32, tag=tag,
                                    bufs=(1 if tag == "py" else 2),
                                    name=f"py{j}_{s}_{nn}")
                    kk = 0
                    for g in range(2):
                        for (lt, rt) in ((oT_hi, wp_h[g]), (oT_lo, wp_h[g]),
                                         (oT_hi, wp_l[g])):
                            nc.tensor.matmul(
                                psy,
                                lt[:, 2 * g:2 * g + 2, :],
                                rt[:, :, nn * 512:(nn + 1) * 512],
                                start=(kk == 0), stop=(kk == 5),
                                perf_mode=DR,
                            )
                            kk += 1
                    ysb = work.tile([128, 512], F32, tag="ysb", bufs=6,
                                    name=f"ys{j}_{s}_{nn}")
                    rows = y[j * QT + s * 128:j * QT + (s + 1) * 128, :]
                    if idx == len(oT_all) - 1:
                        # final drain: halves in parallel on two engines and
                        # two DMA queues to shorten the closing chain
                        nc.scalar.activation(
                            ysb[:, 0:256], psy[:, 0:256],
                            mybir.ActivationFunctionType.Copy,
                        )
                        nc.vector.tensor_copy(ysb[:, 256:512], psy[:, 256:512])
                        nc.sync.dma_start(
                            rows[:, nn * 512:nn * 512 + 256], ysb[:, 0:256]
                        )
                        nc.gpsimd.dma_start(
                            rows[:, nn * 512 + 256:nn * 512 + 512],
                            ysb[:, 256:512],
                        )
                    else:
                        if idx >= 13:
                            nc.scalar.activation(
                                ysb, psy, mybir.ActivationFunctionType.Copy
                            )
                        else:
                            nc.vector.tensor_copy(ysb, psy)
                        yeng = nc.sync if (2 * idx + nn) % 2 == 0 else nc.gpsimd
                        yeng.dma_start(rows[:, nn * 512:(nn + 1) * 512], ysb)

    nc.finalize()
    return nc


_NC = None


def _get_nc():
    global _NC
    if _NC is None:
        _NC = build_kernel()
    return _NC


def _hi_lo(a):
    """Split f32 array into e4m3 hi + lo (returned as ml_dtypes arrays)."""
    import ml_dtypes

    e4 = ml_dtypes.float8_e4m3fn
    hi = a.astype(e4)
    lo = (a - hi.astype(np.float32)).astype(e4)
    return hi, lo


def kernel(x, Wqkv, bqkv, Wproj, bproj, _trace=False):
    x = np.asarray(x, dtype=np.float32)
    Wqkv = np.asarray(Wqkv, dtype=np.float32)
    bqkv = np.asarray(bqkv, dtype=np.float32)
    Wproj = np.asarray(Wproj, dtype=np.float32)
    bproj = np.asarray(bproj, dtype=np.float32)

    import ml_dtypes

    bf16 = ml_dtypes.bfloat16
    # [key, query] diagonal triangle: allow key <= query
    mask = np.triu(np.ones((128, 128), dtype=np.float32)).astype(bf16)
    in_maps = []
    for hg in range(2):
        sl = slice(hg * HGF, (hg + 1) * HGF)
        rows = np.concatenate(
            [Wqkv[sl], Wqkv[1024 + hg * HGF:1024 + (hg + 1) * HGF],
             Wqkv[2048 + hg * HGF:2048 + (hg + 1) * HGF]]
        )
        wqkvT = np.ascontiguousarray(rows.T) * SB          # [C, 1536], 64x
        w_hi, w_lo = _hi_lo(wqkvT)
        # q/k part (cols 0:1024): m-tile pack [1024, 1024]
        #   row = 128*m + p holds W'[256*pr + 128*two + p, 128*m + c]
        def pack_qk(wa):
            blk = wa.astype(np.float32)[:, 0:1024]
            out = np.empty((1024, 1024), dtype=np.float32)
            for m in range(8):
                b4 = blk[:, 128 * m:128 * m + 128].reshape(4, 2, 128, 128)
                out[128 * m:128 * m + 128] = (
                    b4.transpose(2, 0, 1, 3).reshape(128, 1024)
                )
            return out

        # v part (cols 1024:1536): moving pack [512, 1024]
        #   row = 128*pr + p holds W'[256*pr + 128*two + p, 1024 + c]
        def pack_v(wa):
            blk = wa.astype(np.float32)[:, 1024:1536].reshape(4, 2, 128, 512)
            return blk.transpose(0, 2, 1, 3).reshape(512, 1024)

        e4 = ml_dtypes.float8_e4m3fn
        wqkh_np = np.ascontiguousarray(pack_qk(w_hi)).astype(e4)
        wqkl_np = np.ascontiguousarray(pack_qk(w_lo)).astype(e4)
        wvh_np = np.ascontiguousarray(pack_v(w_hi)).astype(e4)
        wvl_np = np.ascontiguousarray(pack_v(w_lo)).astype(e4)
        bq = np.ascontiguousarray(
            (np.concatenate(
                [bqkv[sl], bqkv[1024 + hg * HGF:1024 + (hg + 1) * HGF]]
            ) * (SA * SB)).reshape(8, 128).T
        ).astype(np.float32)
        # proj weights [512, 1024] o-feat rows, 64x; pack [256, 2048]:
        #   row = 128*g + p holds Wp'[256*g + 128*two + p, c]
        wprojT = np.ascontiguousarray(Wproj[:, sl].T) * SB
        wp_hi, wp_lo = _hi_lo(wprojT)

        def pack_wp(wa):
            blk = wa.astype(np.float32).reshape(2, 2, 128, 1024)
            return blk.transpose(0, 2, 1, 3).reshape(256, 2048)

        wph_np = np.ascontiguousarray(pack_wp(wp_hi)).astype(e4)
        wpl_np = np.ascontiguousarray(pack_wp(wp_lo)).astype(e4)
        for b in range(B):
            # x pack: [512, 4096]: row = 128*pr + p,
            # col = 1024*n + 512*two + t holds x'[256*pr + 128*two + p, t]
            xT = np.ascontiguousarray(x[b].T) * SA         # [C, T], 16x
            x_hi, x_lo = _hi_lo(xT)

            def pack_x(xa):
                blk = xa.astype(np.float32).reshape(4, 2, 128, 4, 512)
                return blk.transpose(0, 2, 3, 1, 4).reshape(512, 4096)

            in_maps.append(
                {
                    "xh": np.ascontiguousarray(pack_x(x_hi)).astype(e4),
                    "xl": np.ascontiguousarray(pack_x(x_lo)).astype(e4),
                    "wqkh": wqkh_np,
                    "wqkl": wqkl_np,
                    "wvh": wvh_np,
                    "wvl": wvl_np,
                    "bqk": bq,
                    "wph": wph_np,
                    "wpl": wpl_np,
                    "mask01": mask,
                }
            )
    # core order: idx = hg * 4 + b  (in_maps built hg-major already)
    in_maps = in_maps[:4] + in_maps[4:]
    res = run_bass_kernel_spmd(_get_nc(), in_maps, core_ids=list(range(8)),
                               trace=_trace)
    # V-bias folds into a constant output row: softmax rows sum to 1, so
    # y += (Wproj @ bv) for the full bv (both head groups combined)
    bias_row = bproj + Wproj @ bqkv[2 * C:3 * C]
    out = np.empty((B, T, C), dtype=np.float32)
    for b in range(B):
        out[b] = (res.results[b]["y"] + res.results[4 + b]["y"]) / OUT_DIV \
            + bias_row
    if _trace:
        return out, res
    return out
